# revision 10
# baseline (speedup 1.0000x reference)
"""GCN (3x GraphConv + MLP head) distributed over 8 Trainium2 NeuronCores.

Strategy (graph/data parallel over dst nodes):
  - dst nodes sharded into 8 contiguous slices (6272 nodes/core, padded).
  - Per layer: each core computes its z-slice = (h * rsqrt_deg_out) @ W with
    PE matmuls, casts to bf16, AllGathers the slices so every core holds the
    full z table in HBM, then dma_gathers the source rows for its edges and
    segment-sums them into PSUM via one-hot matmuls (M built on DVE with
    is_equal against an iota).  Eviction applies rsqrt_deg_in scale + bias +
    relu; windows are transposed on PE into feature-major slabs that feed the
    next layer's matmul.
  - MLP head is computed feature-major per core; host transposes the result.

All per-core variation lives in the input data (indices, dst-slot tables,
degree vectors) so one SPMD program serves all 8 cores.
"""

import sys
import numpy as np

sys.path.insert(0, "/opt/trn_rl_repo")

import ml_dtypes  # noqa: E402
import concourse.bass as bass  # noqa: E402
import concourse.bacc as bacc  # noqa: E402
import concourse.mybir as mybir  # noqa: E402
from concourse.tile import TileContext  # noqa: E402
from concourse.bass_utils import run_bass_kernel_spmd  # noqa: E402

F32 = mybir.dt.float32
BF16 = mybir.dt.bfloat16
I16 = mybir.dt.int16
AOP = mybir.AluOpType
ACTF = mybir.ActivationFunctionType

NC = 8
GCHUNK = 8  # tiles per dma_gather (8*128 = 1024 idxs = HW descriptor-ring cap)


def _ceil(a, b):
    return (a + b - 1) // b


def _prep(x, edge_index, W1, b1, W2, b2, W3, b3, mW1, mb1, mW2, mb2):
    """Host-side sharding: returns (cfg, per-core in_maps)."""
    N = x.shape[0]
    slice_n = _ceil(_ceil(N, NC), 128) * 128          # nodes per core (padded)
    npad = slice_n * NC
    half = _ceil(npad // 2, 128) * 128                # lo/hi split of z table
    win = slice_n // 128                              # windows per core

    src = np.asarray(edge_index[0], np.int64)
    dst = np.asarray(edge_index[1], np.int64)
    E = src.shape[0]

    deg_out = np.clip(np.bincount(src, minlength=N), 1, None).astype(np.float32)
    deg_in = np.clip(np.bincount(dst, minlength=N), 1, None).astype(np.float32)
    ro = np.zeros(npad, np.float32)
    ri = np.zeros(npad, np.float32)
    ro[:N] = 1.0 / np.sqrt(deg_out)
    ri[:N] = 1.0 / np.sqrt(deg_in)

    core = dst // slice_n
    wloc = (dst - core * slice_n) // 128
    hi = (src >= half).astype(np.int64)

    # counts per (core, window, pass)
    key = (core * win + wloc) * 2 + hi
    counts = np.bincount(key, minlength=NC * win * 2).reshape(NC, win, 2)
    t_lo = int(_ceil(counts[:, :, 0].max(), 128))
    t_hi = int(_ceil(counts[:, :, 1].max(), 128))
    T = t_lo + t_hi

    # scaled/padded x, transposed per-core
    xs = np.zeros((npad, x.shape[1]), np.float32)
    xs[:N] = np.asarray(x, np.float32) * ro[:N, None]

    ident = np.eye(128, dtype=np.float32)
    iota = np.broadcast_to(
        np.arange(128, dtype=np.float32), (128, T, 128)
    ).astype(ml_dtypes.bfloat16)

    in_maps = []
    order = np.argsort(key, kind="stable")
    starts = np.zeros(NC * win * 2 + 1, np.int64)
    np.cumsum(counts.reshape(-1), out=starts[1:])
    off_in_grp = np.arange(E) - starts[key[order]]

    for c in range(NC):
        lo_src = np.zeros(win * t_lo * 128, np.int64)
        lo_rel = np.full(win * t_lo * 128, -1.0, np.float32)
        hi_src = np.zeros(win * t_hi * 128, np.int64)
        hi_rel = np.full(win * t_hi * 128, -1.0, np.float32)

        sel = order[(core[order] == c)]
        offs = off_in_grp[(core[order] == c)]
        e_src, e_dst, e_w, e_hi = src[sel], dst[sel], wloc[sel], hi[sel]
        e_rel = (e_dst - c * slice_n - e_w * 128).astype(np.float32)
        is_lo = e_hi == 0
        pos_lo = e_w[is_lo] * (t_lo * 128) + offs[is_lo]
        lo_src[pos_lo] = e_src[is_lo]
        lo_rel[pos_lo] = e_rel[is_lo]
        pos_hi = e_w[~is_lo] * (t_hi * 128) + offs[~is_lo]
        hi_src[pos_hi] = e_src[~is_lo] - half
        hi_rel[pos_hi] = e_rel[~is_lo]

        def wrap_idx(vals, t):
            # gather instructions cover chunks of G tiles (G*128 idxs); within
            # each chunk, idx[i] lives at [i%16, i//16] of the chunk's cols
            ntile = win * t
            nchunk = _ceil(ntile, GCHUNK)
            pad = np.zeros(nchunk * GCHUNK * 128, np.int64)
            pad[:ntile * 128] = vals
            blk = pad.reshape(nchunk, GCHUNK * 8, 16)
            out = np.transpose(blk, (2, 0, 1)).reshape(16, nchunk * GCHUNK * 8)
            return np.tile(out.astype(np.int16), (8, 1))

        idx_lo = wrap_idx(lo_src, t_lo)
        idx_hi = wrap_idx(hi_src, t_hi)

        rel_lo = np.transpose(lo_rel.reshape(win, t_lo, 128), (2, 0, 1))
        rel_hi = np.transpose(hi_rel.reshape(win, t_hi, 128), (2, 0, 1))
        dstrel = np.concatenate([rel_lo, rel_hi], axis=2).reshape(128, win * T)
        dstrel = dstrel.astype(ml_dtypes.bfloat16)

        sl = slice(c * slice_n, (c + 1) * slice_n)
        node_ids = np.arange(c * slice_n, (c + 1) * slice_n)
        per_win = node_ids.reshape(win, 128)
        cin12 = (ri * ro)[per_win].T.astype(np.float32).copy()   # [128, win]
        cin3 = ri[per_win].T.astype(np.float32).copy()
        ros = ro[per_win].T.astype(np.float32).copy()

        in_maps.append({
            "xTs": np.ascontiguousarray(xs[sl].T),
            "W1f": np.asarray(W1, np.float32), "W2f": np.asarray(W2, np.float32),
            "W3f": np.asarray(W3, np.float32),
            "b1t": np.broadcast_to(np.asarray(b1, np.float32), (128, 256)).copy(),
            "b2t": np.broadcast_to(np.asarray(b2, np.float32), (128, 256)).copy(),
            "b3t": np.broadcast_to(np.asarray(b3, np.float32), (128, 128)).copy(),
            "mW1f": np.asarray(mW1, np.float32),
            "mW2f": np.asarray(mW2, np.float32),
            "mb1t": np.asarray(mb1, np.float32).reshape(64, 1).copy(),
            "mb2t": np.asarray(mb2, np.float32).reshape(64, 1).copy(),
            "cin12": cin12, "cin3": cin3, "ros": ros,
            "iota": np.ascontiguousarray(iota),
            "ident": ident,
            "idx_lo": np.ascontiguousarray(idx_lo),
            "idx_hi": np.ascontiguousarray(idx_hi),
            "dstrel": np.ascontiguousarray(dstrel),
        })

    cfg = dict(N=N, slice_n=slice_n, npad=npad, half=half, win=win,
               t_lo=t_lo, t_hi=t_hi, T=T,
               nch_lo=_ceil(win * t_lo, GCHUNK), nch_hi=_ceil(win * t_hi, GCHUNK),
               f_in=x.shape[1], f1=W1.shape[1], f2=W2.shape[1], f3=W3.shape[1],
               dm=mW1.shape[1], do=mW2.shape[1])
    return cfg, in_maps


def _build(cfg):
    """Build the SPMD bass program (identical for all cores)."""
    slice_n, npad, half = cfg["slice_n"], cfg["npad"], cfg["half"]
    win, t_lo, t_hi, T = cfg["win"], cfg["t_lo"], cfg["t_hi"], cfg["T"]
    nch_lo, nch_hi = cfg["nch_lo"], cfg["nch_hi"]
    f_in, f1, f2, f3 = cfg["f_in"], cfg["f1"], cfg["f2"], cfg["f3"]
    dm, do = cfg["dm"], cfg["do"]

    nc = bacc.Bacc("TRN2", target_bir_lowering=False, debug=False, num_devices=NC)

    inp = {}
    for name, shape, dt in [
        ("xTs", [f_in, slice_n], F32),
        ("W1f", [f_in, f1], F32), ("W2f", [f1, f2], F32), ("W3f", [f2, f3], F32),
        ("b1t", [128, f1], F32), ("b2t", [128, f2], F32), ("b3t", [128, f3], F32),
        ("mW1f", [f3, dm], F32), ("mW2f", [dm, do], F32),
        ("mb1t", [dm, 1], F32), ("mb2t", [do, 1], F32),
        ("cin12", [128, win], F32), ("cin3", [128, win], F32), ("ros", [128, win], F32),
        ("iota", [128, T, 128], BF16), ("ident", [128, 128], F32),
        ("idx_lo", [128, nch_lo * GCHUNK * 8], I16),
        ("idx_hi", [128, nch_hi * GCHUNK * 8], I16),
        ("dstrel", [128, win * T], BF16),
    ]:
        inp[name] = nc.dram_tensor(name, shape, dt, kind="ExternalInput")

    h_out = nc.dram_tensor("h_out", [slice_n, f3], F32, kind="ExternalOutput")
    yT_out = nc.dram_tensor("yT_out", [do, slice_n], F32, kind="ExternalOutput")

    layers = [
        dict(f=f1, W="W1f", bt="b1t", last=False),
        dict(f=f2, W="W2f", bt="b2t", last=False),
        dict(f=f3, W="W3f", bt="b3t", last=True),
    ]

    with TileContext(nc) as tc:
        with tc.tile_pool(name="const", bufs=1) as cp, \
             tc.tile_pool(name="slab", bufs=2) as slabp, \
             tc.tile_pool(name="glo", bufs=3) as glop, \
             tc.tile_pool(name="ghi", bufs=3) as ghip, \
             tc.tile_pool(name="mb", bufs=3) as mbp, \
             tc.tile_pool(name="ev", bufs=3) as evp, \
             tc.tile_pool(name="zs", bufs=3) as zsp, \
             tc.tile_pool(name="psz", bufs=2, space="PSUM") as pszp, \
             tc.tile_pool(name="psw", bufs=2, space="PSUM") as pswp, \
             tc.tile_pool(name="pst", bufs=2, space="PSUM") as pstp, \
             tc.tile_pool(name="psm", bufs=1, space="PSUM") as psmp, \
             tc.tile_pool(name="dram", bufs=1, space="DRAM") as dram:

            # --- resident constants ---
            def cload(name, shape, dt, src_ap):
                t = cp.tile(shape, dt, tag=name)
                nc.sync.dma_start(t[:], src_ap)
                return t

            Wsb = []
            for li, L in enumerate(layers):
                wt = cp.tile([128, 2, L["f"]], F32, tag=f"W{li}")
                for k in range(2):
                    nc.sync.dma_start(wt[:, k, :], inp[L["W"]][k * 128:(k + 1) * 128, :])
                Wsb.append(wt)
            bts = [cload(L["bt"], [128, L["f"]], F32, inp[L["bt"]][:, :]) for L in layers]
            mW1_t = cload("mW1", [f3, dm], F32, inp["mW1f"][:, :])
            mW2_t = cload("mW2", [dm, do], F32, inp["mW2f"][:, :])
            mb1_t = cload("mb1", [dm, 1], F32, inp["mb1t"][:, :])
            mb2_t = cload("mb2", [do, 1], F32, inp["mb2t"][:, :])
            cin12_t = cload("cin12", [128, win], F32, inp["cin12"][:, :])
            cin3_t = cload("cin3", [128, win], F32, inp["cin3"][:, :])
            ros_t = cload("ros", [128, win], F32, inp["ros"][:, :])
            iota_t = cload("iota", [128, T, 128], BF16, inp["iota"][:, :, :])
            ident_t = cload("ident", [128, 128], F32, inp["ident"][:, :])
            ixlo_t = cload("ixlo", [128, nch_lo * GCHUNK * 8], I16, inp["idx_lo"][:, :])
            ixhi_t = cload("ixhi", [128, nch_hi * GCHUNK * 8], I16, inp["idx_hi"][:, :])
            drel_t = cload("drel", [128, win * T], BF16, inp["dstrel"][:, :])

            # input slab (feature-major x, pre-scaled)
            prev = slabp.tile([128, 2, slice_n], F32, tag="slab")
            for k in range(2):
                nc.sync.dma_start(prev[:, k, :], inp["xTs"][k * 128:(k + 1) * 128, :])

            for li, L in enumerate(layers):
                F = L["f"]
                nko = F // 128  # output feature chunks (2 or 1)

                # ---- z-slice matmul, bf16 cast, store ----
                zslice = dram.tile([slice_n, F], BF16, tag=f"zsl{li}")
                for w in range(win):
                    psz = pszp.tile([128, F], F32)
                    for k in range(2):
                        nc.tensor.matmul(psz[:], prev[:, k, w * 128:(w + 1) * 128],
                                         Wsb[li][:, k, :],
                                         start=(k == 0), stop=(k == 1))
                    zt = zsp.tile([128, F], BF16)
                    nc.scalar.copy(zt[:], psz[:])
                    nc.sync.dma_start(zslice[w * 128:(w + 1) * 128, :], zt[:])

                ztab = dram.tile([npad, F], BF16, tag=f"ztab{li}")
                nc.gpsimd.collective_compute(
                    "AllGather", AOP.bypass,
                    replica_groups=[list(range(NC))],
                    ins=[zslice.opt()], outs=[ztab.opt()])

                # ---- aggregation ----
                newslab = slabp.tile([128, 2, slice_n], F32, tag="slab")
                z_lo = ztab[0:half, :]
                z_hi = ztab[half:npad, :]
                # gather chunk tiles, emitted lazily in stream order
                chunk_tiles = {}

                def get_chunk(p, c):
                    if (p, c) in chunk_tiles:
                        return chunk_tiles[(p, c)]
                    pool, ix, zsrc = ((glop, ixlo_t, z_lo) if p == 0
                                      else (ghip, ixhi_t, z_hi))
                    g = pool.tile([128, GCHUNK, F], BF16)
                    nc.gpsimd.dma_gather(
                        g[:], zsrc, ix[:, c * GCHUNK * 8:(c + 1) * GCHUNK * 8],
                        GCHUNK * 128, GCHUNK * 128, F)
                    chunk_tiles[(p, c)] = g
                    return g

                for w in range(win):
                    mt = mbp.tile([128, T, 128], BF16)
                    nc.vector.tensor_tensor(
                        mt[:], iota_t[:],
                        drel_t[:, w * T:(w + 1) * T, None].broadcast_to([128, T, 128]),
                        AOP.is_equal)
                    psw = pswp.tile([128, F], F32)
                    for j in range(T):
                        p, jl, tp = (0, j, t_lo) if j < t_lo else (1, j - t_lo, t_hi)
                        c, s = divmod(w * tp + jl, GCHUNK)
                        rhs = get_chunk(p, c)[:, s, :]
                        nc.tensor.matmul(psw[:], mt[:, j, :], rhs,
                                         start=(j == 0), stop=(j == T - 1))

                    # ---- eviction ----
                    hw1 = evp.tile([128, F], F32, tag="hw1")
                    if not L["last"]:
                        bso = evp.tile([128, F], F32, tag="bso")
                        nc.vector.tensor_scalar(
                            bso[:], bts[li][:], ros_t[:, w:w + 1], None, AOP.mult)
                        nc.vector.scalar_tensor_tensor(
                            hw1[:], psw[:], cin12_t[:, w:w + 1], bso[:],
                            AOP.mult, AOP.add)
                    else:
                        nc.vector.scalar_tensor_tensor(
                            hw1[:], psw[:], cin3_t[:, w:w + 1], bts[li][:],
                            AOP.mult, AOP.add)
                    hw2 = evp.tile([128, F], F32, tag="hw2")
                    nc.vector.tensor_scalar(hw2[:], hw1[:], 0.0, None, AOP.max)
                    if L["last"]:
                        nc.sync.dma_start(h_out[w * 128:(w + 1) * 128, :], hw2[:])
                    for k in range(nko):
                        pst = pstp.tile([128, 128], F32)
                        nc.tensor.transpose(pst[:], hw2[:, k * 128:(k + 1) * 128],
                                            ident_t[:])
                        nc.vector.tensor_copy(newslab[:, k, w * 128:(w + 1) * 128],
                                              pst[:])
                prev = newslab

            # ---- MLP head (feature-major) ----
            chunks = []
            a = 0
            while a < slice_n:
                L_ = min(512, slice_n - a)
                chunks.append((a, L_))
                a += L_
            for (a, L_) in chunks:
                ps1 = psmp.tile([dm, 512], F32, tag="ps1")
                nc.tensor.matmul(ps1[:, :L_], mW1_t[:], prev[:, 0, a:a + L_],
                                 start=True, stop=True)
                z1 = zsp.tile([dm, 512], F32, tag="z1")
                nc.scalar.activation(z1[:, :L_], ps1[:, :L_], ACTF.Relu,
                                     bias=mb1_t[:])
                ps2 = psmp.tile([do, 512], F32, tag="ps2")
                nc.tensor.matmul(ps2[:, :L_], mW2_t[:], z1[:, :L_],
                                 start=True, stop=True)
                y = zsp.tile([do, 512], F32, tag="y")
                nc.scalar.activation(y[:, :L_], ps2[:, :L_], ACTF.Identity,
                                     bias=mb2_t[:])
                nc.sync.dma_start(yT_out[:, a:a + L_], y[:, :L_])

    nc.compile()
    return nc


_CACHE = {}


def kernel(x, edge_index, W1, b1, W2, b2, W3, b3, mW1, mb1, mW2, mb2,
           _want_results=False, _trace=False):
    x = np.asarray(x)
    cfg, in_maps = _prep(x, np.asarray(edge_index), W1, b1, W2, b2, W3, b3,
                         mW1, mb1, mW2, mb2)
    key = (cfg["N"], cfg["t_lo"], cfg["t_hi"], cfg["f_in"], cfg["f1"],
           cfg["f2"], cfg["f3"], cfg["dm"], cfg["do"])
    if key not in _CACHE:
        _CACHE[key] = _build(cfg)
    nc = _CACHE[key]

    kw = {}
    if _trace:
        kw = dict(trace=True, stitch_traces=False)
    res = run_bass_kernel_spmd(nc, in_maps, core_ids=list(range(NC)), **kw)

    N = cfg["N"]
    h_parts = [r["h_out"] for r in res.results]
    y_parts = [r["yT_out"].T for r in res.results]
    h_last = np.concatenate(h_parts, 0)[:N]
    out = np.concatenate(y_parts, 0)[:N]
    if _want_results:
        return (out, h_last), res
    return (out, h_last)


# revision 14
# speedup vs baseline: 1.0720x; 1.0720x over previous
"""GCN (3x GraphConv + MLP head) distributed over 8 Trainium2 NeuronCores.

Strategy (graph/data parallel over dst nodes):
  - dst nodes sharded into 8 contiguous slices (6272 nodes/core, padded).
  - Per layer: each core computes its z-slice = (h * rsqrt_deg_out) @ W with
    PE matmuls (bf16), AllGathers the slices so every core holds the full z
    table in HBM, then dma_gathers the source rows for its edges and
    segment-sums them into PSUM via one-hot matmuls (M built on DVE with
    is_equal against an iota).  Eviction applies rsqrt_deg_in scale + bias +
    relu; windows are transposed on PE into feature-major slabs that feed the
    next layer's matmul.
  - The AllGather is split in two (first/second half of every core's slice,
    giving the "lo"/"hi" z tables, each 25088 rows so gather indices fit
    int16).  z production for layer l+1 is emitted inline inside layer l's
    aggregation loop so AG-A(l+1) fires mid-layer and is fully hidden; lo
    gathers run ahead of hi gathers so AG-B's tail overlaps real work.
  - MLP head is computed feature-major per core; host transposes the result.

All per-core variation lives in the input data (indices, dst-slot tables,
degree vectors) so one SPMD program serves all 8 cores.
"""

import sys
import numpy as np

sys.path.insert(0, "/opt/trn_rl_repo")

import ml_dtypes  # noqa: E402
import concourse.bass as bass  # noqa: E402
import concourse.bacc as bacc  # noqa: E402
import concourse.mybir as mybir  # noqa: E402
from concourse.tile import TileContext  # noqa: E402
from concourse.bass_utils import run_bass_kernel_spmd  # noqa: E402

F32 = mybir.dt.float32
BF16 = mybir.dt.bfloat16
I16 = mybir.dt.int16
AOP = mybir.AluOpType
ACTF = mybir.ActivationFunctionType

NC = 8
GCHUNK = 8   # tiles per dma_gather (8*128 = 1024 idxs = HW descriptor-ring cap)
KA = 6       # lo-gather lookahead (windows) to hide the AG-B tail


def _ceil(a, b):
    return (a + b - 1) // b


def _prep(x, edge_index, W1, b1, W2, b2, W3, b3, mW1, mb1, mW2, mb2):
    """Host-side sharding: returns (cfg, per-core in_maps)."""
    N = x.shape[0]
    slice_n = _ceil(_ceil(N, NC), 256) * 256           # nodes per core (padded)
    npad = slice_n * NC
    halfs = slice_n // 2                               # slice half (lo/hi tables)
    win = slice_n // 128                               # windows per core

    src = np.asarray(edge_index[0], np.int64)
    dst = np.asarray(edge_index[1], np.int64)
    E = src.shape[0]

    deg_out = np.clip(np.bincount(src, minlength=N), 1, None).astype(np.float32)
    deg_in = np.clip(np.bincount(dst, minlength=N), 1, None).astype(np.float32)
    ro = np.zeros(npad, np.float32)
    ri = np.zeros(npad, np.float32)
    ro[:N] = 1.0 / np.sqrt(deg_out)
    ri[:N] = 1.0 / np.sqrt(deg_in)

    core = dst // slice_n
    wloc = (dst - core * slice_n) // 128
    sc = src // slice_n
    sj = src - sc * slice_n
    hi = (sj >= halfs).astype(np.int64)
    row_lo = sc * halfs + sj                 # valid when hi == 0
    row_hi = sc * halfs + (sj - halfs)       # valid when hi == 1

    key = (core * win + wloc) * 2 + hi
    counts = np.bincount(key, minlength=NC * win * 2).reshape(NC, win, 2)
    t_lo = int(_ceil(counts[:, :, 0].max(), 128))
    t_hi = int(_ceil(counts[:, :, 1].max(), 128))
    T = t_lo + t_hi

    xs = np.zeros((npad, x.shape[1]), np.float32)
    xs[:N] = np.asarray(x, np.float32) * ro[:N, None]

    identf = np.eye(128, dtype=np.float32)
    identb = np.eye(128, dtype=np.float32).astype(ml_dtypes.bfloat16)
    iota = np.broadcast_to(
        np.arange(128, dtype=np.float32), (128, T, 128)
    ).astype(ml_dtypes.bfloat16)

    in_maps = []
    order = np.argsort(key, kind="stable")
    starts = np.zeros(NC * win * 2 + 1, np.int64)
    np.cumsum(counts.reshape(-1), out=starts[1:])
    off_in_grp = np.arange(E) - starts[key[order]]

    table_row = np.where(hi == 0, row_lo, row_hi)

    for c in range(NC):
        lo_src = np.zeros(win * t_lo * 128, np.int64)
        lo_rel = np.full(win * t_lo * 128, -1.0, np.float32)
        hi_src = np.zeros(win * t_hi * 128, np.int64)
        hi_rel = np.full(win * t_hi * 128, -1.0, np.float32)

        msk = core[order] == c
        sel = order[msk]
        offs = off_in_grp[msk]
        e_row, e_dst, e_w, e_hi = table_row[sel], dst[sel], wloc[sel], hi[sel]
        e_rel = (e_dst - c * slice_n - e_w * 128).astype(np.float32)
        is_lo = e_hi == 0
        pos_lo = e_w[is_lo] * (t_lo * 128) + offs[is_lo]
        lo_src[pos_lo] = e_row[is_lo]
        lo_rel[pos_lo] = e_rel[is_lo]
        pos_hi = e_w[~is_lo] * (t_hi * 128) + offs[~is_lo]
        hi_src[pos_hi] = e_row[~is_lo]
        hi_rel[pos_hi] = e_rel[~is_lo]

        def wrap_idx(vals, t):
            # gather instructions cover chunks of GCHUNK tiles; within each
            # chunk, idx[i] lives at [i%16, i//16] of the chunk's columns
            ntile = win * t
            nchunk = _ceil(ntile, GCHUNK)
            pad = np.zeros(nchunk * GCHUNK * 128, np.int64)
            pad[:ntile * 128] = vals
            blk = pad.reshape(nchunk, GCHUNK * 8, 16)
            out = np.transpose(blk, (2, 0, 1)).reshape(16, nchunk * GCHUNK * 8)
            return np.tile(out.astype(np.int16), (8, 1))

        idx_lo = wrap_idx(lo_src, t_lo)
        idx_hi = wrap_idx(hi_src, t_hi)

        rel_lo = np.transpose(lo_rel.reshape(win, t_lo, 128), (2, 0, 1))
        rel_hi = np.transpose(hi_rel.reshape(win, t_hi, 128), (2, 0, 1))
        dstrel = np.concatenate([rel_lo, rel_hi], axis=2).reshape(128, win * T)
        dstrel = dstrel.astype(ml_dtypes.bfloat16)

        sl = slice(c * slice_n, (c + 1) * slice_n)
        per_win = np.arange(c * slice_n, (c + 1) * slice_n).reshape(win, 128)
        cin12 = (ri * ro)[per_win].T.astype(np.float32).copy()
        cin3 = ri[per_win].T.astype(np.float32).copy()
        ros = ro[per_win].T.astype(np.float32).copy()

        in_maps.append({
            "xTs": np.ascontiguousarray(xs[sl].T.astype(ml_dtypes.bfloat16)),
            "W1f": np.asarray(W1, np.float32).astype(ml_dtypes.bfloat16),
            "W2f": np.asarray(W2, np.float32).astype(ml_dtypes.bfloat16),
            "W3f": np.asarray(W3, np.float32).astype(ml_dtypes.bfloat16),
            "b1t": np.broadcast_to(np.asarray(b1, np.float32), (128, 256)).copy(),
            "b2t": np.broadcast_to(np.asarray(b2, np.float32), (128, 256)).copy(),
            "b3t": np.broadcast_to(np.asarray(b3, np.float32), (128, 128)).copy(),
            "mW1f": np.asarray(mW1, np.float32).astype(ml_dtypes.bfloat16),
            "mW2f": np.asarray(mW2, np.float32).astype(ml_dtypes.bfloat16),
            "mb1t": np.asarray(mb1, np.float32).reshape(64, 1).copy(),
            "mb2t": np.asarray(mb2, np.float32).reshape(64, 1).copy(),
            "cin12": cin12, "cin3": cin3, "ros": ros,
            "iota": np.ascontiguousarray(iota),
            "identf": identf, "identb": identb,
            "idx_lo": np.ascontiguousarray(idx_lo),
            "idx_hi": np.ascontiguousarray(idx_hi),
            "dstrel": np.ascontiguousarray(dstrel),
        })

    cfg = dict(N=N, slice_n=slice_n, npad=npad, halfs=halfs, win=win,
               t_lo=t_lo, t_hi=t_hi, T=T,
               nch_lo=_ceil(win * t_lo, GCHUNK), nch_hi=_ceil(win * t_hi, GCHUNK),
               f_in=x.shape[1], f1=W1.shape[1], f2=W2.shape[1], f3=W3.shape[1],
               dm=mW1.shape[1], do=mW2.shape[1])
    return cfg, in_maps


def _build(cfg):
    """Build the SPMD bass program (identical for all cores)."""
    slice_n, npad, halfs = cfg["slice_n"], cfg["npad"], cfg["halfs"]
    win, t_lo, t_hi, T = cfg["win"], cfg["t_lo"], cfg["t_hi"], cfg["T"]
    nch_lo, nch_hi = cfg["nch_lo"], cfg["nch_hi"]
    f_in, f1, f2, f3 = cfg["f_in"], cfg["f1"], cfg["f2"], cfg["f3"]
    dm, do = cfg["dm"], cfg["do"]
    trows = NC * halfs

    nc = bacc.Bacc("TRN2", target_bir_lowering=False, debug=False, num_devices=NC)

    inp = {}
    for name, shape, dt in [
        ("xTs", [f_in, slice_n], BF16),
        ("W1f", [f_in, f1], BF16), ("W2f", [f1, f2], BF16), ("W3f", [f2, f3], BF16),
        ("b1t", [128, f1], F32), ("b2t", [128, f2], F32), ("b3t", [128, f3], F32),
        ("mW1f", [f3, dm], BF16), ("mW2f", [dm, do], BF16),
        ("mb1t", [dm, 1], F32), ("mb2t", [do, 1], F32),
        ("cin12", [128, win], F32), ("cin3", [128, win], F32), ("ros", [128, win], F32),
        ("iota", [128, T, 128], BF16),
        ("identf", [128, 128], F32), ("identb", [128, 128], BF16),
        ("idx_lo", [128, nch_lo * GCHUNK * 8], I16),
        ("idx_hi", [128, nch_hi * GCHUNK * 8], I16),
        ("dstrel", [128, win * T], BF16),
    ]:
        inp[name] = nc.dram_tensor(name, shape, dt, kind="ExternalInput")

    h_out = nc.dram_tensor("h_out", [slice_n, f3], F32, kind="ExternalOutput")
    yT_out = nc.dram_tensor("yT_out", [do, slice_n], F32, kind="ExternalOutput")

    layers = [
        dict(f=f1, W="W1f", bt="b1t", last=False),
        dict(f=f2, W="W2f", bt="b2t", last=False),
        dict(f=f3, W="W3f", bt="b3t", last=True),
    ]

    with TileContext(nc) as tc:
        with tc.tile_pool(name="const", bufs=1) as cp, \
             tc.tile_pool(name="slab", bufs=2) as slabp, \
             tc.tile_pool(name="glo", bufs=9) as glop, \
             tc.tile_pool(name="ghi", bufs=3) as ghip, \
             tc.tile_pool(name="mb", bufs=3) as mbp, \
             tc.tile_pool(name="ev", bufs=3) as evp, \
             tc.tile_pool(name="zs", bufs=3) as zsp, \
             tc.tile_pool(name="psz", bufs=2, space="PSUM") as pszp, \
             tc.tile_pool(name="psw", bufs=3, space="PSUM") as pswp, \
             tc.tile_pool(name="pst", bufs=1, space="PSUM") as pstp, \
             tc.tile_pool(name="psm", bufs=1, space="PSUM") as psmp, \
             tc.tile_pool(name="dram", bufs=1, space="DRAM") as dram:

            def cload(name, shape, dt, src_ap):
                t = cp.tile(shape, dt, tag=name)
                nc.sync.dma_start(t[:], src_ap)
                return t

            Wsb = []
            for li, L in enumerate(layers):
                wt = cp.tile([128, 2, L["f"]], BF16, tag=f"W{li}")
                for k in range(2):
                    nc.sync.dma_start(wt[:, k, :], inp[L["W"]][k * 128:(k + 1) * 128, :])
                Wsb.append(wt)
            bts = [cload(L["bt"], [128, L["f"]], F32, inp[L["bt"]][:, :]) for L in layers]
            mW1_t = cload("mW1", [f3, dm], BF16, inp["mW1f"][:, :])
            mW2_t = cload("mW2", [dm, do], BF16, inp["mW2f"][:, :])
            mb1_t = cload("mb1", [dm, 1], F32, inp["mb1t"][:, :])
            mb2_t = cload("mb2", [do, 1], F32, inp["mb2t"][:, :])
            cin12_t = cload("cin12", [128, win], F32, inp["cin12"][:, :])
            cin3_t = cload("cin3", [128, win], F32, inp["cin3"][:, :])
            ros_t = cload("ros", [128, win], F32, inp["ros"][:, :])
            iota_t = cload("iota", [128, T, 128], BF16, inp["iota"][:, :, :])
            identf_t = cload("identf", [128, 128], F32, inp["identf"][:, :])
            identb_t = cload("identb", [128, 128], BF16, inp["identb"][:, :])
            ixlo_t = cload("ixlo", [128, nch_lo * GCHUNK * 8], I16, inp["idx_lo"][:, :])
            ixhi_t = cload("ixhi", [128, nch_hi * GCHUNK * 8], I16, inp["idx_hi"][:, :])
            drel_t = cload("drel", [128, win * T], BF16, inp["dstrel"][:, :])

            xslab = slabp.tile([128, 2, slice_n], BF16, tag="slab")
            for k in range(2):
                nc.sync.dma_start(xslab[:, k, :], inp["xTs"][k * 128:(k + 1) * 128, :])

            # dram buffers per layer
            zslices, ztabsA, ztabsB = [], [], []
            for li, L in enumerate(layers):
                zslices.append(dram.tile([slice_n, L["f"]], BF16,
                                         tag=f"zsl{li}", name=f"zsl{li}"))
                ztabsA.append(dram.tile([trows, L["f"]], BF16,
                                        tag=f"ztA{li}", name=f"ztA{li}"))
                ztabsB.append(dram.tile([trows, L["f"]], BF16,
                                        tag=f"ztB{li}", name=f"ztB{li}"))

            rg = [list(range(NC))]

            def emit_zmm(li, w, src_slab):
                """z-slice window w for layer li from its input slab."""
                F = layers[li]["f"]
                psz = pszp.tile([128, F], F32, tag="psz")
                for k in range(2):
                    nc.tensor.matmul(psz[:], src_slab[:, k, w * 128:(w + 1) * 128],
                                     Wsb[li][:, k, :], start=(k == 0), stop=(k == 1))
                zt = zsp.tile([128, F], BF16, tag="zsb")
                nc.scalar.copy(zt[:], psz[:])
                nc.sync.dma_start(zslices[li][w * 128:(w + 1) * 128, :], zt[:])

            def emit_ag(li, part):
                F = layers[li]["f"]
                if part == 0:
                    nc.gpsimd.collective_compute(
                        "AllGather", AOP.bypass, replica_groups=rg,
                        ins=[zslices[li][0:halfs, :]], outs=[ztabsA[li].opt()])
                else:
                    nc.gpsimd.collective_compute(
                        "AllGather", AOP.bypass, replica_groups=rg,
                        ins=[zslices[li][halfs:slice_n, :]], outs=[ztabsB[li].opt()])

            # layer-0 z production from the input slab
            for w in range(win):
                emit_zmm(0, w, xslab)
                if w == win // 2:
                    emit_ag(0, 0)
            emit_ag(0, 1)

            prev = xslab
            for li, L in enumerate(layers):
                F = L["f"]
                nko = F // 128

                newslab = slabp.tile([128, 2, slice_n], BF16, tag="slab")
                chunk_tiles = {}
                emitted = [0, 0]

                def get_chunk(p, c, li=li, chunk_tiles=chunk_tiles):
                    pool, ix, zsrc = ((glop, ixlo_t, ztabsA[li]) if p == 0
                                      else (ghip, ixhi_t, ztabsB[li]))
                    g = pool.tile([128, GCHUNK, F], BF16,
                                  tag="glo" if p == 0 else "ghi")
                    nc.gpsimd.dma_gather(
                        g[:], zsrc[:, :], ix[:, c * GCHUNK * 8:(c + 1) * GCHUNK * 8],
                        GCHUNK * 128, GCHUNK * 128, F)
                    chunk_tiles[(p, c)] = g
                    return g

                def ensure(p, upto_chunk, emitted=emitted):
                    while emitted[p] <= upto_chunk:
                        get_chunk(p, emitted[p])
                        emitted[p] += 1

                for w in range(win):
                    wl = min(w + KA, win - 1)
                    ensure(0, ((wl + 1) * t_lo - 1) // GCHUNK)
                    ensure(1, ((w + 1) * t_hi - 1) // GCHUNK)

                    mt = mbp.tile([128, T, 128], BF16, tag="mt")
                    nc.vector.tensor_tensor(
                        mt[:], iota_t[:],
                        drel_t[:, w * T:(w + 1) * T, None].broadcast_to([128, T, 128]),
                        AOP.is_equal)
                    psw = pswp.tile([128, F], F32, tag="psw")
                    for j in range(T):
                        p, jl, tp = (0, j, t_lo) if j < t_lo else (1, j - t_lo, t_hi)
                        c, s = divmod(w * tp + jl, GCHUNK)
                        rhs = chunk_tiles[(p, c)][:, s, :]
                        nc.tensor.matmul(psw[:], mt[:, j, :], rhs,
                                         start=(j == 0), stop=(j == T - 1))

                    # ---- eviction ----
                    hw1 = evp.tile([128, F], F32, tag="hw1")
                    if not L["last"]:
                        bso = evp.tile([128, F], F32, tag="bso")
                        nc.vector.tensor_scalar(
                            bso[:], bts[li][:], ros_t[:, w:w + 1], None, AOP.mult)
                        nc.vector.scalar_tensor_tensor(
                            hw1[:], psw[:], cin12_t[:, w:w + 1], bso[:],
                            AOP.mult, AOP.add)
                        hw2 = evp.tile([128, F], BF16, tag="hw2b")
                        ident = identb_t
                        pdt, ptag = BF16, "pstb"
                    else:
                        nc.vector.scalar_tensor_tensor(
                            hw1[:], psw[:], cin3_t[:, w:w + 1], bts[li][:],
                            AOP.mult, AOP.add)
                        hw2 = evp.tile([128, F], F32, tag="hw2f")
                        ident = identf_t
                        pdt, ptag = F32, "pstf"
                    nc.vector.tensor_scalar(hw2[:], hw1[:], 0.0, None, AOP.max)
                    if L["last"]:
                        nc.sync.dma_start(h_out[w * 128:(w + 1) * 128, :], hw2[:])
                    for k in range(nko):
                        pst = pstp.tile([128, 128], pdt, tag=ptag)
                        nc.tensor.transpose(pst[:], hw2[:, k * 128:(k + 1) * 128],
                                            ident[:])
                        nc.vector.tensor_copy(newslab[:, k, w * 128:(w + 1) * 128],
                                              pst[:])
                    # inline z production for the next layer
                    if li < 2:
                        emit_zmm(li + 1, w, newslab)
                        if w == min(win // 2, win - 2):
                            emit_ag(li + 1, 0)
                        if w == win - 1:
                            emit_ag(li + 1, 1)
                prev = newslab

            # ---- MLP head (feature-major) ----
            a = 0
            while a < slice_n:
                L_ = min(512, slice_n - a)
                ps1 = psmp.tile([dm, 512], F32, tag="psm")
                nc.tensor.matmul(ps1[:, :L_], mW1_t[:], prev[:, 0, a:a + L_],
                                 start=True, stop=True)
                z1 = zsp.tile([dm, 512], BF16, tag="z1")
                nc.scalar.activation(z1[:, :L_], ps1[:, :L_], ACTF.Relu,
                                     bias=mb1_t[:])
                ps2 = psmp.tile([do, 512], F32, tag="psm")
                nc.tensor.matmul(ps2[:, :L_], mW2_t[:], z1[:, :L_],
                                 start=True, stop=True)
                y = zsp.tile([do, 512], F32, tag="y")
                nc.scalar.activation(y[:, :L_], ps2[:, :L_], ACTF.Identity,
                                     bias=mb2_t[:])
                nc.sync.dma_start(yT_out[:, a:a + L_], y[:, :L_])
                a += L_

    nc.compile()
    return nc


_CACHE = {}


def kernel(x, edge_index, W1, b1, W2, b2, W3, b3, mW1, mb1, mW2, mb2,
           _want_results=False, _trace=False):
    x = np.asarray(x)
    cfg, in_maps = _prep(x, np.asarray(edge_index), W1, b1, W2, b2, W3, b3,
                         mW1, mb1, mW2, mb2)
    key = (cfg["N"], cfg["t_lo"], cfg["t_hi"], cfg["f_in"], cfg["f1"],
           cfg["f2"], cfg["f3"], cfg["dm"], cfg["do"])
    if key not in _CACHE:
        _CACHE[key] = _build(cfg)
    nc = _CACHE[key]

    kw = {}
    if _trace:
        kw = dict(trace=True, stitch_traces=False)
    res = run_bass_kernel_spmd(nc, in_maps, core_ids=list(range(NC)), **kw)

    N = cfg["N"]
    h_parts = [r["h_out"] for r in res.results]
    y_parts = [r["yT_out"].T for r in res.results]
    h_last = np.concatenate(h_parts, 0)[:N]
    out = np.concatenate(y_parts, 0)[:N]
    if _want_results:
        return (out, h_last), res
    return (out, h_last)
